# revision 5
# baseline (speedup 1.0000x reference)
"""Trainium2 Bass kernel for blocked multi-head attention + layernorm.

Reference semantics (B=4, S=8192, D=1024, H=16):
    q/k/v = x @ w{q,k,v}                 [B, S, D]
    reshape to [B*H, S/H, D] = [64, 512, 1024]  (sequence-split blocks)
    scores = q @ k^T * (D//H)**-0.5      per block, [64, 512, 512]
    attn   = softmax(scores, axis=-1)
    ctx    = attn @ v                    [64, 512, 1024]
    out    = layernorm(query + ctx @ w_final^T + b_final) * gamma + beta
    returns (out [4, 8192, 1024], attn [64, 512, 512])

Sharding: data-parallel over the 64 independent blocks, 8 blocks per core on
8 NeuronCores.  Weights replicated.  No cross-core traffic.

Per-core schedule (software-pipelined across blocks to keep TensorE dense):
    proj(n) -> scores(n)+softmax -> final(n-1) -> context(n) -> ...
attn transposes for the context matmul run on the DMA engines (bf16 XBAR),
keeping the PE stream pure matmul.
"""

import sys

for _p in ("/opt/trn_rl_repo", "/root/.axon_site/_ro/trn_rl_repo"):
    if _p not in sys.path:
        sys.path.append(_p)

import numpy as np
import ml_dtypes

import concourse.bass as bass
import concourse.tile as tile
from concourse import bacc, mybir
from concourse.bass import ds
from concourse.bass_utils import run_bass_kernel_spmd

F32 = mybir.dt.float32
BF16 = mybir.dt.bfloat16

B, S, D, H = 4, 8192, 1024, 16
BLK = S // H            # 512 rows per attention block
NBLK = B * H            # 64 blocks total
NCORES = 8
BPC = NBLK // NCORES    # 8 blocks per core
SCALE = (D // H) ** -0.5
EPS = 1e-5
P = 128                 # SBUF partitions
KZ = D // P             # 8 contraction chunks
IZ = BLK // P           # 4 row chunks per block
NF = 512                # matmul moving free dim / PSUM bank in fp32


def _build_module():
    nc = bacc.Bacc(None)

    qT_d = nc.dram_tensor("qT", [BPC, D, BLK], BF16, kind="ExternalInput")
    kT_d = nc.dram_tensor("kT", [BPC, D, BLK], BF16, kind="ExternalInput")
    vT_d = nc.dram_tensor("vT", [BPC, D, BLK], BF16, kind="ExternalInput")
    res_d = nc.dram_tensor("res", [BPC, BLK, D], F32, kind="ExternalInput")
    wq_d = nc.dram_tensor("wq", [D, D], BF16, kind="ExternalInput")
    wk_d = nc.dram_tensor("wk", [D, D], BF16, kind="ExternalInput")
    wv_d = nc.dram_tensor("wv", [D, D], BF16, kind="ExternalInput")
    wfT_d = nc.dram_tensor("wfT", [D, D], BF16, kind="ExternalInput")
    gamma_d = nc.dram_tensor("gamma", [D], F32, kind="ExternalInput")
    beta_d = nc.dram_tensor("beta", [D], F32, kind="ExternalInput")
    out_d = nc.dram_tensor("out", [BPC, BLK, D], F32, kind="ExternalOutput")
    attn_d = nc.dram_tensor("attn", [BPC, BLK, BLK], F32, kind="ExternalOutput")

    with tile.TileContext(nc) as tc:
        with (
            tc.tile_pool(name="singles", bufs=1) as singles,
            tc.tile_pool(name="inpq", bufs=2) as inpq_pool,
            tc.tile_pool(name="inpkv", bufs=1) as inpkv_pool,
            tc.tile_pool(name="blk", bufs=1) as blkpool,
            tc.tile_pool(name="ct", bufs=2) as ct_pool,
            tc.tile_pool(name="attnf", bufs=3) as attn_pool,
            tc.tile_pool(name="resp", bufs=2) as res_pool,
            tc.tile_pool(name="xp", bufs=2) as x_pool,
            tc.tile_pool(name="yp", bufs=2) as y_pool,
            tc.tile_pool(name="stat", bufs=4) as stat_pool,
            tc.tile_pool(name="psmm", bufs=8, space="PSUM") as psum_mm,
        ):
            # persistent tiles (loaded lazily inside block 0)
            wq_sb = singles.tile([P, KZ, D], BF16, tag="wq")
            wk_sb = singles.tile([P, KZ, D], BF16, tag="wk")
            wv_sb = singles.tile([P, KZ, D], BF16, tag="wv")
            wfT_sb = singles.tile([P, KZ, D], BF16, tag="wfT")
            gamma_bc = singles.tile([P, D], F32, tag="gamma")
            beta_bc = singles.tile([P, D], F32, tag="beta")
            eps_t = singles.tile([P, 1], F32, tag="eps")

            def final_phase(blk, cT_sb):
                if blk == 0:
                    wfT_r = wfT_d[:, :].rearrange("(ko p) d -> p ko d", p=P)
                    for kz in range(KZ):
                        nc.sync.dma_start(wfT_sb[:, kz, :], wfT_r[:, kz, :])
                    for v_bc, v_d in ((gamma_bc, gamma_d), (beta_bc, beta_d)):
                        g = v_d[:]
                        nc.gpsimd.dma_start(
                            out=v_bc,
                            in_=bass.AP(tensor=g.tensor, offset=g.offset,
                                        ap=[[0, P]] + list(g.ap)),
                        )
                    nc.vector.memset(eps_t, EPS)
                for iz in range(IZ):
                    res_t = res_pool.tile([P, D], F32, tag="res")
                    nc.sync.dma_start(res_t, res_d[blk, ds(iz * P, P), :])
                    x = x_pool.tile([P, D], F32, tag="x")
                    for mz in range(D // NF):
                        ps = psum_mm.tile([P, NF], F32, tag="mm")
                        for dz in range(KZ):
                            nc.tensor.matmul(
                                ps,
                                lhsT=cT_sb[:, dz, ds(iz * P, P)],
                                rhs=wfT_sb[:, dz, ds(mz * NF, NF)],
                                start=(dz == 0),
                                stop=(dz == KZ - 1),
                            )
                        nc.vector.tensor_add(
                            x[:, ds(mz * NF, NF)], ps, res_t[:, ds(mz * NF, NF)]
                        )
                    stats = stat_pool.tile([P, 2, 6], F32, tag="bnst")
                    for sg in range(2):
                        nc.vector.bn_stats(out=stats[:, sg, :], in_=x[:, ds(sg * NF, NF)])
                    mv = stat_pool.tile([P, 2], F32, tag="mv")
                    nc.vector.bn_aggr(out=mv, in_=stats)
                    rstd = stat_pool.tile([P, 1], F32, tag="rstd")
                    nc.scalar.activation(
                        out=rstd, in_=mv[:, 1:2],
                        func=mybir.ActivationFunctionType.Sqrt, bias=eps_t,
                    )
                    nc.vector.reciprocal(rstd, rstd)
                    y = y_pool.tile([P, D], F32, tag="y")
                    nc.vector.tensor_scalar(
                        out=y, in0=x, scalar1=mv[:, 0:1], scalar2=rstd,
                        op0=mybir.AluOpType.subtract, op1=mybir.AluOpType.mult,
                    )
                    nc.vector.tensor_mul(y, y, gamma_bc)
                    nc.vector.tensor_add(y, y, beta_bc)
                    nc.sync.dma_start(out_d[blk, ds(iz * P, P), :], y)

            pending_final = None
            for blk in range(BPC):
                # ---- load inputs (feature-major) ----
                xq = inpq_pool.tile([P, KZ, BLK], BF16, tag="xq")
                xk = inpkv_pool.tile([P, KZ, BLK], BF16, tag="xk")
                xv = inpkv_pool.tile([P, KZ, BLK], BF16, tag="xv")
                if blk == 0:
                    # stagger startup: first matmul starts after one chunk
                    qT_r = qT_d[blk].rearrange("(ko p) i -> p ko i", p=P)
                    wq_r = wq_d[:, :].rearrange("(ko p) d -> p ko d", p=P)
                    for kz in range(KZ):
                        nc.sync.dma_start(wq_sb[:, kz, :], wq_r[:, kz, :])
                        nc.sync.dma_start(xq[:, kz, :], qT_r[:, kz, :])
                else:
                    nc.sync.dma_start(xq, qT_d[blk].rearrange("(ko p) i -> p ko i", p=P))
                    nc.sync.dma_start(xk, kT_d[blk].rearrange("(ko p) i -> p ko i", p=P))
                    nc.sync.dma_start(xv, vT_d[blk].rearrange("(ko p) i -> p ko i", p=P))

                # ---- q/k projections (feature-major outputs) ----
                qT_sb = blkpool.tile([P, KZ, BLK], BF16, tag="qT")
                kT_sb = blkpool.tile([P, KZ, BLK], BF16, tag="kT")
                for ti, (x_sb, w_sb, o_sb) in enumerate(
                    ((xq, wq_sb, qT_sb), (xk, wk_sb, kT_sb))
                ):
                    if blk == 0 and ti == 1:
                        kT_r = kT_d[blk].rearrange("(ko p) i -> p ko i", p=P)
                        wk_r = wk_d[:, :].rearrange("(ko p) d -> p ko d", p=P)
                        for kz in range(KZ):
                            nc.sync.dma_start(wk_sb[:, kz, :], wk_r[:, kz, :])
                            nc.sync.dma_start(xk[:, kz, :], kT_r[:, kz, :])
                    for dz in range(KZ):
                        ps = psum_mm.tile([P, NF], F32, tag="mm")
                        for kz in range(KZ):
                            nc.tensor.matmul(
                                ps,
                                lhsT=w_sb[:, kz, ds(dz * P, P)],
                                rhs=x_sb[:, kz, :],
                                start=(kz == 0),
                                stop=(kz == KZ - 1),
                            )
                        nc.scalar.copy(o_sb[:, dz, :], ps)

                # ---- v projection (row-major output) ----
                if blk == 0:
                    vT_r = vT_d[blk].rearrange("(ko p) i -> p ko i", p=P)
                    wv_r = wv_d[:, :].rearrange("(ko p) d -> p ko d", p=P)
                    for kz in range(KZ):
                        nc.sync.dma_start(wv_sb[:, kz, :], wv_r[:, kz, :])
                        nc.sync.dma_start(xv[:, kz, :], vT_r[:, kz, :])
                v_sb = blkpool.tile([P, IZ, D], BF16, tag="v")
                for jz in range(IZ):
                    for dh in range(D // NF):
                        ps = psum_mm.tile([P, NF], F32, tag="mm")
                        for kz in range(KZ):
                            nc.tensor.matmul(
                                ps,
                                lhsT=xv[:, kz, ds(jz * P, P)],
                                rhs=wv_sb[:, kz, ds(dh * NF, NF)],
                                start=(kz == 0),
                                stop=(kz == KZ - 1),
                            )
                        nc.scalar.copy(v_sb[:, jz, ds(dh * NF, NF)], ps)

                # ---- scores + softmax; attn^T built by DMA transposes ----
                aT_sb = blkpool.tile([P, IZ, BLK], BF16, tag="aT")
                for iz in range(IZ):
                    ps = psum_mm.tile([P, NF], F32, tag="mm")
                    for dz in range(KZ):
                        nc.tensor.matmul(
                            ps,
                            lhsT=qT_sb[:, dz, ds(iz * P, P)],
                            rhs=kT_sb[:, dz, :],
                            start=(dz == 0),
                            stop=(dz == KZ - 1),
                        )
                    nmax = stat_pool.tile([P, 1], F32, tag="nmax")
                    nc.vector.tensor_reduce(
                        out=nmax, in_=ps, axis=mybir.AxisListType.X,
                        op=mybir.AluOpType.max, negate=True,
                    )
                    nc.vector.tensor_scalar_mul(nmax, nmax, SCALE)
                    e = attn_pool.tile([P, BLK], F32, tag="attnf")
                    rsum = stat_pool.tile([P, 1], F32, tag="rsum")
                    nc.scalar.activation(
                        out=e, in_=ps, func=mybir.ActivationFunctionType.Exp,
                        bias=nmax, scale=SCALE, accum_out=rsum,
                    )
                    rinv = stat_pool.tile([P, 1], F32, tag="rinv")
                    nc.vector.reciprocal(rinv, rsum)
                    # normalized fp32 for the attention output (ScalarE)...
                    a32 = attn_pool.tile([P, BLK], F32, tag="a32")
                    nc.scalar.activation(
                        out=a32, in_=e, func=mybir.ActivationFunctionType.Copy,
                        scale=rinv,
                    )
                    nc.sync.dma_start(attn_d[blk, ds(iz * P, P), :], a32)
                    # ...normalized bf16 for the context matmul (VectorE)
                    abf = attn_pool.tile([P, BLK], BF16, tag="abf")
                    nc.vector.tensor_scalar_mul(abf, e, rinv)
                    for jz in range(IZ):
                        nc.sync.dma_start(
                            aT_sb[:, jz, ds(iz * P, P)],
                            abf[:, ds(jz * P, P)],
                            transpose=True,
                        )

                # ---- pipelined: previous block's output proj + layernorm ----
                if pending_final is not None:
                    final_phase(*pending_final)

                # ---- context^T = (attn @ v)^T ----
                cT_sb = ct_pool.tile([P, KZ, BLK], BF16, tag="cT")
                for dz in range(KZ):
                    ps = psum_mm.tile([P, NF], F32, tag="mm")
                    for jz in range(IZ):
                        nc.tensor.matmul(
                            ps,
                            lhsT=v_sb[:, jz, ds(dz * P, P)],
                            rhs=aT_sb[:, jz, :],
                            start=(jz == 0),
                            stop=(jz == IZ - 1),
                        )
                    nc.scalar.copy(cT_sb[:, dz, :], ps)

                pending_final = (blk, cT_sb)

            final_phase(*pending_final)

    nc.compile()
    return nc


_MODULE = None


def _get_module():
    global _MODULE
    if _MODULE is None:
        _MODULE = _build_module()
    return _MODULE


def _prep_inputs(query, key, value, wq, wk, wv, w_final, b_final, gamma, beta):
    """Host-side sharding + layout prep (not part of HW kernel time)."""
    bf16 = ml_dtypes.bfloat16
    qb = np.ascontiguousarray(np.asarray(query, np.float32).reshape(NBLK, BLK, D))
    kb = np.asarray(key, np.float32).reshape(NBLK, BLK, D)
    vb = np.asarray(value, np.float32).reshape(NBLK, BLK, D)

    qT = np.ascontiguousarray(qb.transpose(0, 2, 1)).astype(bf16)
    kT = np.ascontiguousarray(kb.transpose(0, 2, 1)).astype(bf16)
    vT = np.ascontiguousarray(vb.transpose(0, 2, 1)).astype(bf16)
    res = qb + np.asarray(b_final, np.float32)[None, None, :]

    wq_bf = np.asarray(wq, np.float32).astype(bf16)
    wk_bf = np.asarray(wk, np.float32).astype(bf16)
    wv_bf = np.asarray(wv, np.float32).astype(bf16)
    wfT_bf = np.ascontiguousarray(np.asarray(w_final, np.float32).T).astype(bf16)
    gamma_f = np.ascontiguousarray(np.asarray(gamma, np.float32))
    beta_f = np.ascontiguousarray(np.asarray(beta, np.float32))

    in_maps = []
    for c in range(NCORES):
        sl = slice(c * BPC, (c + 1) * BPC)
        in_maps.append({
            "qT": np.ascontiguousarray(qT[sl]),
            "kT": np.ascontiguousarray(kT[sl]),
            "vT": np.ascontiguousarray(vT[sl]),
            "res": np.ascontiguousarray(res[sl]),
            "wq": wq_bf, "wk": wk_bf, "wv": wv_bf, "wfT": wfT_bf,
            "gamma": gamma_f, "beta": beta_f,
        })
    return in_maps


def kernel(query, key, value, wq, wk, wv, w_final, b_final, gamma, beta,
           _trace=False, _trace_kwargs=None):
    nc = _get_module()
    in_maps = _prep_inputs(query, key, value, wq, wk, wv, w_final, b_final,
                           gamma, beta)
    kw = {}
    if _trace:
        kw = {"trace": True}
        if _trace_kwargs:
            kw.update(_trace_kwargs)
    res = run_bass_kernel_spmd(nc, in_maps, core_ids=list(range(NCORES)), **kw)

    out = np.empty((NBLK, BLK, D), np.float32)
    attn = np.empty((NBLK, BLK, BLK), np.float32)
    for c in range(NCORES):
        sl = slice(c * BPC, (c + 1) * BPC)
        out[sl] = res.results[c]["out"]
        attn[sl] = res.results[c]["attn"]

    kernel.last_results = res
    return out.reshape(B, S, D), attn


# revision 9
# speedup vs baseline: 1.1932x; 1.1932x over previous
"""Trainium2 Bass kernel for blocked multi-head attention + layernorm.

Reference semantics (B=4, S=8192, D=1024, H=16):
    q/k/v = x @ w{q,k,v}                 [B, S, D]
    reshape to [B*H, S/H, D] = [64, 512, 1024]  (sequence-split blocks)
    scores = q @ k^T * (D//H)**-0.5      per block, [64, 512, 512]
    attn   = softmax(scores, axis=-1)
    ctx    = attn @ v                    [64, 512, 1024]
    out    = layernorm(query + ctx @ w_final^T + b_final) * gamma + beta
    returns (out [4, 8192, 1024], attn [64, 512, 512])

Sharding: data-parallel over the 64 independent blocks, 8 blocks per core on
8 NeuronCores.  Weights replicated.  No cross-core traffic.

Per-core schedule (software-pipelined across blocks to keep TensorE dense):
    proj(n) -> scores(n)+softmax -> final(n-1) -> context(n) -> ...
attn transposes for the context matmul run on the DMA engines (bf16 XBAR),
keeping the PE stream pure matmul.
"""

import sys

for _p in ("/opt/trn_rl_repo", "/root/.axon_site/_ro/trn_rl_repo"):
    if _p not in sys.path:
        sys.path.append(_p)

import numpy as np
import ml_dtypes

import concourse.bass as bass
import concourse.tile as tile
from concourse import bacc, mybir
from concourse.bass import ds
from concourse.bass_utils import run_bass_kernel_spmd
from concourse.masks import make_identity

F32 = mybir.dt.float32
BF16 = mybir.dt.bfloat16

B, S, D, H = 4, 8192, 1024, 16
BLK = S // H            # 512 rows per attention block
NBLK = B * H            # 64 blocks total
NCORES = 8
BPC = NBLK // NCORES    # 8 blocks per core
SCALE = (D // H) ** -0.5
EPS = 1e-5
P = 128                 # SBUF partitions
KZ = D // P             # 8 contraction chunks
IZ = BLK // P           # 4 row chunks per block
NF = 512                # matmul moving free dim / PSUM bank in fp32


def _build_module():
    nc = bacc.Bacc(None)

    qT_d = nc.dram_tensor("qT", [BPC, D, BLK], BF16, kind="ExternalInput")
    kT_d = nc.dram_tensor("kT", [BPC, D, BLK], BF16, kind="ExternalInput")
    vT_d = nc.dram_tensor("vT", [BPC, D, BLK], BF16, kind="ExternalInput")
    res_d = nc.dram_tensor("res", [BPC, BLK, D], F32, kind="ExternalInput")
    wq_d = nc.dram_tensor("wq", [D, D], BF16, kind="ExternalInput")
    wk_d = nc.dram_tensor("wk", [D, D], BF16, kind="ExternalInput")
    wv_d = nc.dram_tensor("wv", [D, D], BF16, kind="ExternalInput")
    wfT_d = nc.dram_tensor("wfT", [D, D], BF16, kind="ExternalInput")
    gamma_d = nc.dram_tensor("gamma", [D], F32, kind="ExternalInput")
    beta_d = nc.dram_tensor("beta", [D], F32, kind="ExternalInput")
    out_d = nc.dram_tensor("out", [BPC, BLK, D], F32, kind="ExternalOutput")
    attn_d = nc.dram_tensor("attn", [BPC, BLK, BLK], F32, kind="ExternalOutput")

    with tile.TileContext(nc) as tc:
        with (
            tc.tile_pool(name="singles", bufs=1) as singles,
            tc.tile_pool(name="inpq", bufs=2) as inpq_pool,
            tc.tile_pool(name="inpkv", bufs=1) as inpkv_pool,
            tc.tile_pool(name="blk", bufs=1) as blkpool,
            tc.tile_pool(name="ct", bufs=2) as ct_pool,
            tc.tile_pool(name="attnf", bufs=3) as attn_pool,
            tc.tile_pool(name="resp", bufs=2) as res_pool,
            tc.tile_pool(name="xp", bufs=2) as x_pool,
            tc.tile_pool(name="yp", bufs=2) as y_pool,
            tc.tile_pool(name="stat", bufs=4) as stat_pool,
            tc.tile_pool(name="psmm", bufs=6, space="PSUM") as psum_mm,
            tc.tile_pool(name="pstp", bufs=2, space="PSUM") as psum_tp,
        ):
            # persistent tiles (loaded lazily inside block 0)
            wq_sb = singles.tile([P, KZ, D], BF16, tag="wq")
            wk_sb = singles.tile([P, KZ, D], BF16, tag="wk")
            wv_sb = singles.tile([P, KZ, D], BF16, tag="wv")
            wfT_sb = singles.tile([P, KZ, D], BF16, tag="wfT")
            gamma_bc = singles.tile([P, D], F32, tag="gamma")
            beta_bc = singles.tile([P, D], F32, tag="beta")
            eps_t = singles.tile([P, 1], F32, tag="eps")
            ident = singles.tile([P, P], F32, tag="ident")

            def final_phase(blk, cT_sb):
                if blk == 0:
                    wfT_r = wfT_d[:, :].rearrange("(ko p) d -> p ko d", p=P)
                    for kz in range(KZ):
                        nc.sync.dma_start(wfT_sb[:, kz, :], wfT_r[:, kz, :])
                    for v_bc, v_d in ((gamma_bc, gamma_d), (beta_bc, beta_d)):
                        g = v_d[:]
                        nc.gpsimd.dma_start(
                            out=v_bc,
                            in_=bass.AP(tensor=g.tensor, offset=g.offset,
                                        ap=[[0, P]] + list(g.ap)),
                        )
                    nc.vector.memset(eps_t, EPS)
                for iz in range(IZ):
                    res_t = res_pool.tile([P, D], F32, tag="res")
                    nc.sync.dma_start(res_t, res_d[blk, ds(iz * P, P), :])
                    x = x_pool.tile([P, D], F32, tag="x")
                    for mz in range(D // NF):
                        ps = psum_mm.tile([P, NF], F32, tag="mm")
                        for dz in range(KZ):
                            nc.tensor.matmul(
                                ps,
                                lhsT=cT_sb[:, dz, ds(iz * P, P)],
                                rhs=wfT_sb[:, dz, ds(mz * NF, NF)],
                                start=(dz == 0),
                                stop=(dz == KZ - 1),
                            )
                        nc.vector.tensor_add(
                            x[:, ds(mz * NF, NF)], ps, res_t[:, ds(mz * NF, NF)]
                        )
                    stats = stat_pool.tile([P, 2, 6], F32, tag="bnst")
                    for sg in range(2):
                        nc.vector.bn_stats(out=stats[:, sg, :], in_=x[:, ds(sg * NF, NF)])
                    mv = stat_pool.tile([P, 2], F32, tag="mv")
                    nc.vector.bn_aggr(out=mv, in_=stats)
                    rstd = stat_pool.tile([P, 1], F32, tag="rstd")
                    nc.scalar.activation(
                        out=rstd, in_=mv[:, 1:2],
                        func=mybir.ActivationFunctionType.Sqrt, bias=eps_t,
                    )
                    nc.vector.reciprocal(rstd, rstd)
                    y = y_pool.tile([P, D], F32, tag="y")
                    nc.vector.tensor_scalar(
                        out=y, in0=x, scalar1=mv[:, 0:1], scalar2=rstd,
                        op0=mybir.AluOpType.subtract, op1=mybir.AluOpType.mult,
                    )
                    nc.vector.tensor_mul(y, y, gamma_bc)
                    nc.vector.tensor_add(y, y, beta_bc)
                    nc.sync.dma_start(out_d[blk, ds(iz * P, P), :], y)

            pending_final = None
            for blk in range(BPC):
                # ---- load inputs (feature-major) ----
                xq = inpq_pool.tile([P, KZ, BLK], BF16, tag="xq")
                xk = inpkv_pool.tile([P, KZ, BLK], BF16, tag="xk")
                xv = inpkv_pool.tile([P, KZ, BLK], BF16, tag="xv")
                if blk == 0:
                    # stagger startup: first matmul starts after one chunk
                    qT_r = qT_d[blk].rearrange("(ko p) i -> p ko i", p=P)
                    wq_r = wq_d[:, :].rearrange("(ko p) d -> p ko d", p=P)
                    for kz in range(KZ):
                        nc.sync.dma_start(wq_sb[:, kz, :], wq_r[:, kz, :])
                        nc.sync.dma_start(xq[:, kz, :], qT_r[:, kz, :])
                else:
                    nc.sync.dma_start(xq, qT_d[blk].rearrange("(ko p) i -> p ko i", p=P))
                    nc.sync.dma_start(xk, kT_d[blk].rearrange("(ko p) i -> p ko i", p=P))
                    nc.sync.dma_start(xv, vT_d[blk].rearrange("(ko p) i -> p ko i", p=P))

                # ---- q/k projections (feature-major outputs) ----
                qT_sb = blkpool.tile([P, KZ, BLK], BF16, tag="qT")
                kT_sb = blkpool.tile([P, KZ, BLK], BF16, tag="kT")
                for ti, (x_sb, w_sb, o_sb) in enumerate(
                    ((xq, wq_sb, qT_sb), (xk, wk_sb, kT_sb))
                ):
                    if blk == 0 and ti == 1:
                        kT_r = kT_d[blk].rearrange("(ko p) i -> p ko i", p=P)
                        wk_r = wk_d[:, :].rearrange("(ko p) d -> p ko d", p=P)
                        for kz in range(KZ):
                            nc.sync.dma_start(wk_sb[:, kz, :], wk_r[:, kz, :])
                            nc.sync.dma_start(xk[:, kz, :], kT_r[:, kz, :])
                    for dz in range(KZ):
                        ps = psum_mm.tile([P, NF], F32, tag="mm")
                        for kz in range(KZ):
                            nc.tensor.matmul(
                                ps,
                                lhsT=w_sb[:, kz, ds(dz * P, P)],
                                rhs=x_sb[:, kz, :],
                                start=(kz == 0),
                                stop=(kz == KZ - 1),
                            )
                        nc.scalar.copy(o_sb[:, dz, :], ps)

                # ---- v projection (row-major output) ----
                if blk == 0:
                    vT_r = vT_d[blk].rearrange("(ko p) i -> p ko i", p=P)
                    wv_r = wv_d[:, :].rearrange("(ko p) d -> p ko d", p=P)
                    for kz in range(KZ):
                        nc.sync.dma_start(wv_sb[:, kz, :], wv_r[:, kz, :])
                        nc.sync.dma_start(xv[:, kz, :], vT_r[:, kz, :])
                v_sb = blkpool.tile([P, IZ, D], BF16, tag="v")
                for jz in range(IZ):
                    for dh in range(D // NF):
                        ps = psum_mm.tile([P, NF], F32, tag="mm")
                        for kz in range(KZ):
                            nc.tensor.matmul(
                                ps,
                                lhsT=xv[:, kz, ds(jz * P, P)],
                                rhs=wv_sb[:, kz, ds(dh * NF, NF)],
                                start=(kz == 0),
                                stop=(kz == KZ - 1),
                            )
                        nc.scalar.copy(v_sb[:, jz, ds(dh * NF, NF)], ps)

                # ---- scores + softmax ----
                if blk == 0:
                    make_identity(nc, ident)
                aT_sb = blkpool.tile([P, IZ, BLK], BF16, tag="aT")
                a32_tiles = []
                for iz in range(IZ):
                    ps = psum_mm.tile([P, NF], F32, tag="mm")
                    for dz in range(KZ):
                        nc.tensor.matmul(
                            ps,
                            lhsT=qT_sb[:, dz, ds(iz * P, P)],
                            rhs=kT_sb[:, dz, :],
                            start=(dz == 0),
                            stop=(dz == KZ - 1),
                        )
                    nmax = stat_pool.tile([P, 1], F32, tag="nmax")
                    nc.vector.tensor_reduce(
                        out=nmax, in_=ps, axis=mybir.AxisListType.X,
                        op=mybir.AluOpType.max, negate=True,
                    )
                    nc.vector.tensor_scalar_mul(nmax, nmax, SCALE)
                    e = attn_pool.tile([P, BLK], F32, tag="attnf")
                    rsum = stat_pool.tile([P, 1], F32, tag="rsum")
                    nc.scalar.activation(
                        out=e, in_=ps, func=mybir.ActivationFunctionType.Exp,
                        bias=nmax, scale=SCALE, accum_out=rsum,
                    )
                    rinv = stat_pool.tile([P, 1], F32, tag="rinv")
                    nc.vector.reciprocal(rinv, rsum)
                    a32 = attn_pool.tile([P, BLK], F32, tag="a32")
                    nc.scalar.activation(
                        out=a32, in_=e, func=mybir.ActivationFunctionType.Copy,
                        scale=rinv,
                    )
                    nc.sync.dma_start(attn_d[blk, ds(iz * P, P), :], a32)
                    a32_tiles.append(a32)

                # ---- pipelined: previous block's output proj + layernorm ----
                # (runs on PE while this block's softmax chain completes)
                if pending_final is not None:
                    final_phase(*pending_final)

                # ---- attn^T via PE transposes (softmax long done by now) ----
                for iz in range(IZ):
                    for jz in range(IZ):
                        pt = psum_tp.tile([P, P], F32, tag="tp")
                        nc.tensor.transpose(pt, a32_tiles[iz][:, ds(jz * P, P)], ident)
                        nc.scalar.copy(aT_sb[:, jz, ds(iz * P, P)], pt)

                # ---- context^T = (attn @ v)^T ----
                cT_sb = ct_pool.tile([P, KZ, BLK], BF16, tag="cT")
                for dz in range(KZ):
                    ps = psum_mm.tile([P, NF], F32, tag="mm")
                    for jz in range(IZ):
                        nc.tensor.matmul(
                            ps,
                            lhsT=v_sb[:, jz, ds(dz * P, P)],
                            rhs=aT_sb[:, jz, :],
                            start=(jz == 0),
                            stop=(jz == IZ - 1),
                        )
                    nc.scalar.copy(cT_sb[:, dz, :], ps)

                pending_final = (blk, cT_sb)

            final_phase(*pending_final)

    nc.compile()
    return nc


_MODULE = None


def _get_module():
    global _MODULE
    if _MODULE is None:
        _MODULE = _build_module()
    return _MODULE


def _prep_inputs(query, key, value, wq, wk, wv, w_final, b_final, gamma, beta):
    """Host-side sharding + layout prep (not part of HW kernel time)."""
    bf16 = ml_dtypes.bfloat16
    qb = np.ascontiguousarray(np.asarray(query, np.float32).reshape(NBLK, BLK, D))
    kb = np.asarray(key, np.float32).reshape(NBLK, BLK, D)
    vb = np.asarray(value, np.float32).reshape(NBLK, BLK, D)

    qT = np.ascontiguousarray(qb.transpose(0, 2, 1)).astype(bf16)
    kT = np.ascontiguousarray(kb.transpose(0, 2, 1)).astype(bf16)
    vT = np.ascontiguousarray(vb.transpose(0, 2, 1)).astype(bf16)
    res = qb + np.asarray(b_final, np.float32)[None, None, :]

    wq_bf = np.asarray(wq, np.float32).astype(bf16)
    wk_bf = np.asarray(wk, np.float32).astype(bf16)
    wv_bf = np.asarray(wv, np.float32).astype(bf16)
    wfT_bf = np.ascontiguousarray(np.asarray(w_final, np.float32).T).astype(bf16)
    gamma_f = np.ascontiguousarray(np.asarray(gamma, np.float32))
    beta_f = np.ascontiguousarray(np.asarray(beta, np.float32))

    in_maps = []
    for c in range(NCORES):
        sl = slice(c * BPC, (c + 1) * BPC)
        in_maps.append({
            "qT": np.ascontiguousarray(qT[sl]),
            "kT": np.ascontiguousarray(kT[sl]),
            "vT": np.ascontiguousarray(vT[sl]),
            "res": np.ascontiguousarray(res[sl]),
            "wq": wq_bf, "wk": wk_bf, "wv": wv_bf, "wfT": wfT_bf,
            "gamma": gamma_f, "beta": beta_f,
        })
    return in_maps


def kernel(query, key, value, wq, wk, wv, w_final, b_final, gamma, beta,
           _trace=False, _trace_kwargs=None):
    nc = _get_module()
    in_maps = _prep_inputs(query, key, value, wq, wk, wv, w_final, b_final,
                           gamma, beta)
    kw = {}
    if _trace:
        kw = {"trace": True}
        if _trace_kwargs:
            kw.update(_trace_kwargs)
    res = run_bass_kernel_spmd(nc, in_maps, core_ids=list(range(NCORES)), **kw)

    out = np.empty((NBLK, BLK, D), np.float32)
    attn = np.empty((NBLK, BLK, BLK), np.float32)
    for c in range(NCORES):
        sl = slice(c * BPC, (c + 1) * BPC)
        out[sl] = res.results[c]["out"]
        attn[sl] = res.results[c]["attn"]

    kernel.last_results = res
    return out.reshape(B, S, D), attn
